# revision 1
# baseline (speedup 1.0000x reference)
"""Grouped SwiGLU MoE (16 experts, top-2, 8192x1024 tokens, d_ff 2816) on 8 TRN2 cores.

Expert-parallel, 2 expert slots per core. Host does the integer routing (sort
tokens by expert) and rounds all matmul operands to bf16 (fp32 PSUM
accumulation keeps the result well inside the 2e-2 gate). Experts are PAIRED
largest-with-smallest across cores so the two static per-slot capacities
(cap0 >= cap1) hug the actual token counts -- minimal padding FLOPs under the
SPMD one-program constraint.

Per core, per expert slot, the device computes in feature-major layout
  yT = w3^T ( silu(w1^T xg) * (w2^T xv) ),   xv = x * route_weight
so y leaves the w3 contraction already route-weighted and the writeout is a
bare DMA. w1/w2/w3 stream from HBM in f-tile groups [2,4,...,4] while
x / y_acc stay SBUF-resident. Slot1 reuses slot0's x tiles (WAR-gated
refills; the big slice is double-buffered so its refill runs a group early),
y_acc is per-slot so slot0's single writeout DMA drains under slot1's
compute, and output DMAs ride the Activation-engine DGE queue so they never
contend with the SP-queue weight stream. Host un-shards and sums the two
expert contributions per token.
"""

import numpy as np

N_EXPERTS, D_MODEL, D_FF = 16, 1024, 2816
N_TOKENS, TOP_K = 8192, 2
N_CORES = 8
E_LOCAL = N_EXPERTS // N_CORES  # 2 expert slots per core
DK = D_MODEL // 128             # 8 contraction tiles for x
FK = D_FF // 128                # 22 f tiles
F_GROUP = 4                     # f-tiles per streamed weight group

_MM_DT = "bfloat16"             # matmul dtype: bf16 = fast LS + half DMA;
                                # ("float32r" = 11-bit mantissa, "float32" exact)

# ----------------------------------------------------------------- host utils


def _to_f32r(x: np.ndarray) -> np.ndarray:
    """Round fp32 -> fp32r bits (RNE to 11 explicit mantissa bits).

    Matches the hardware's fp32->fp32r cast bit-for-bit (verified against a
    DVE tensor_copy on device).
    """
    u = np.ascontiguousarray(x, dtype=np.float32).view(np.uint32).astype(np.uint64)
    r = (u + np.uint64(0x7FF) + ((u >> np.uint64(12)) & np.uint64(1))) & ~np.uint64(0xFFF)
    return r.astype(np.uint32).view(np.float32).reshape(x.shape)


def _slice_plan(cap: int) -> list[int]:
    """Split cap (multiple of 8) into matmul free-dim pieces, all >=256 when
    possible (stays above the ~235-row LoadStationary cadence floor, and
    fp32r needs >=256 for full rate), smallest first so the kernel head
    needs the least data before the first matmul."""
    if cap <= 512:
        return [cap]
    out = []
    rem = cap
    while rem > 1024:
        out.append(512)
        rem -= 512
    if rem > 512:
        a = ((rem // 2 + 7) // 8) * 8
        out += [rem - a, a]
    else:
        out.append(rem)
    return sorted(out)


def _sub_plan(plan0: list[int], cap1: int) -> list[int]:
    """Slot1 slice widths fitted under slot0's slice tiles."""
    out, rem = [], cap1
    for p in plan0:
        w = min(p, rem)
        if w <= 0:
            break
        out.append(w)
        rem -= w
    assert rem == 0, (plan0, cap1)
    return out


# ------------------------------------------------------- walrus wait-split fix


def _split_excess_waits(nc):
    """This walrus build encodes at most ONE sync wait per instruction; Tile
    can attach several (first matmul of a group, kernel-tail drain). Hoist the
    excess into standalone InstEventSemaphore (the shape wait_ge emits)."""
    import bass_rust
    import concourse.mybir as mybir

    n = 0
    for fn in nc.m.functions:
        for blk in fn.blocks:
            out, changed = [], False
            for inst in blk.instructions:
                si = inst.sync_info
                if si is not None and si.on_wait is not None and len(si.on_wait) > 1:
                    waits = list(si.on_wait)
                    for w in waits[:-1]:
                        ev = mybir.InstEventSemaphore(name=f"I-wsplit-{n}", ins=[], outs=[])
                        n += 1
                        ev.engine = inst.engine
                        ev.sync_info = bass_rust.SyncInfo(on_wait=[w], on_update=[])
                        out.append(ev)
                    inst.sync_info = bass_rust.SyncInfo(
                        on_wait=waits[-1:], on_update=list(si.on_update or [])
                    )
                    changed = True
                out.append(inst)
            if changed:
                blk.instructions = out
    return n


# ------------------------------------------------------------- device program


def _build(cap0: int, cap1: int):
    import concourse.bass as bass
    import concourse.tile as tile
    import concourse.mybir as mybir

    f32 = mybir.dt.float32
    f32r = getattr(mybir.dt, _MM_DT)
    plan0 = _slice_plan(cap0)
    plans = [list(enumerate(_starts(plan0))), list(enumerate(_starts(_sub_plan(plan0, cap1))))]
    caps = [cap0, cap1]

    # Group sizes [2,4,...,4]: the first group keeps the head's critical
    # weight transfer small; later groups are big (fewer y_acc updates on
    # vector) and tensor-heavy enough that writeout work hides underneath.
    if FK > 6 and (FK - 2) % 4 == 0:
        sizes = [2] + [4] * ((FK - 2) // 4)
    else:
        sizes, rem = [], FK
        while rem:
            g = min(F_GROUP, rem)
            sizes.append(g)
            rem -= g
    groups = []
    f0 = 0
    for g in sizes:
        groups.append((f0, g))
        f0 += g
    NG = len(groups)

    nc = bass.Bass()
    # xg: tokens for the gate matmul; xv: the same tokens pre-scaled by the
    # routing weight, fed to the value matmul -- y then comes out of the w3
    # contraction already route-weighted, so the writeout is a bare DMA.
    xgt = [
        nc.dram_tensor(f"xt{e}", [DK, 128, caps[e]], f32r, kind="ExternalInput")
        for e in range(E_LOCAL)
    ]
    xvt = [
        nc.dram_tensor(f"xv{e}", [DK, 128, caps[e]], f32r, kind="ExternalInput")
        for e in range(E_LOCAL)
    ]
    w1t = nc.dram_tensor("w1t", [E_LOCAL, DK, 128, D_FF], f32r, kind="ExternalInput")
    w2t = nc.dram_tensor("w2t", [E_LOCAL, DK, 128, D_FF], f32r, kind="ExternalInput")
    w3t = nc.dram_tensor("w3t", [E_LOCAL, FK, 128, D_MODEL], f32r, kind="ExternalInput")
    yt = [
        nc.dram_tensor(f"yt{e}", [DK, 128, caps[e]], f32, kind="ExternalOutput")
        for e in range(E_LOCAL)
    ]

    with tile.TileContext(nc) as tc:
        with (
            tc.tile_pool(name="xts", bufs=1) as p_x,
            tc.tile_pool(name="xbig", bufs=2) as p_x2,
            tc.tile_pool(name="w12", bufs=2) as p_w12,
            tc.tile_pool(name="wh", bufs=1) as p_wh,
            tc.tile_pool(name="w3", bufs=2) as p_w3,
            tc.tile_pool(name="hs", bufs=2 * F_GROUP) as p_hs,
            tc.tile_pool(name="sil", bufs=3) as p_sil,
            tc.tile_pool(name="yacc", bufs=1) as p_y,
            tc.tile_pool(name="gv", bufs=4, space="PSUM") as p_gv,
            tc.tile_pool(name="py", bufs=4, space="PSUM") as p_py,
        ):
            def dma_xg(e, tiles, si, s0, w):
                # single descriptor-issue per slice (sync-engine issue slots
                # are ~0.8us each; 8 per-dk issues would serialize the head)
                nc.sync.dma_start(
                    out=tiles[si][:, :, :w],
                    in_=xgt[e][:, :, s0:s0 + w].rearrange("a p f -> p a f"),
                )

            def dma_xv(e, tiles, si, s0, w):
                nc.sync.dma_start(
                    out=tiles[si][:, :, :w],
                    in_=xvt[e][:, :, s0:s0 + w].rearrange("a p f -> p a f"),
                )

            def dma_w12(e, f0, glen):
                fw = glen * 128
                w1r = p_w12.tile([128, DK, F_GROUP * 128], f32r, tag="w1r")
                w2r = p_w12.tile([128, DK, F_GROUP * 128], f32r, tag="w2r")
                nc.sync.dma_start(
                    out=w1r[:, :, :fw],
                    in_=w1t[e, :, :, f0 * 128:f0 * 128 + fw].rearrange("a p f -> p a f"),
                )
                nc.sync.dma_start(
                    out=w2r[:, :, :fw],
                    in_=w2t[e, :, :, f0 * 128:f0 * 128 + fw].rearrange("a p f -> p a f"),
                )
                return w1r, w2r

            def dma_w3(e, f0, glen):
                w3r = p_w3.tile([128, F_GROUP, DK, 128], f32r, tag="w3r")
                nc.sync.dma_start(
                    out=w3r[:, :glen, :, :],
                    in_=w3t[e, f0:f0 + glen].rearrange("a p (b d) -> p a b d", b=DK),
                )
                return w3r

            # ---- slot0 preamble: x slice0 + w1/w2 g0 first (the critical
            # path to the first matmul chain), then the rest.
            # Last (largest) slice lives in a double-buffered pool so slot1's
            # refill needs no WAR wait and can run a whole group early.
            n_sl = len(plans[0])
            big_si = n_sl - 1

            def alloc_x(kind, si, w):
                pool = p_x2 if si == big_si else p_x
                return pool.tile(
                    [128, DK, w], f32r, tag=f"{kind}{si}", name=f"{kind}{si}"
                )

            xg = [alloc_x("xg", si, w) for si, (s0, w) in plans[0]]
            xv = [alloc_x("xv", si, w) for si, (s0, w) in plans[0]]
            xg1, xv1 = list(xg), list(xv)  # slot1 views; big slice swapped
            big_early = len(plans[1]) == n_sl

            # Group-0 weights arrive as per-half tiles: the first g-chain can
            # start after xg slice0 + half of w1 (~1MB) instead of the full
            # w1/w2 pair.
            g0len = groups[0][1]
            h0 = DK // 2
            w12_g0 = []
            dma_xg(0, xg, 0, 0, plans[0][0][1][1])
            for nm, wt_ in (("w1", w1t), ("w2", w2t)):
                halves = []
                for half in range(2):
                    ht = p_wh.tile(
                        [128, h0, F_GROUP * 128], f32r,
                        tag=f"{nm}h{half}", name=f"{nm}h{half}",
                    )
                    halves.append(ht)
                w12_g0.append(halves)
                if nm == "w1":
                    nc.sync.dma_start(
                        out=halves[0][:, :, :g0len * 128],
                        in_=w1t[0, :h0, :, :g0len * 128].rearrange("a p f -> p a f"),
                    )
                    dma_xv(0, xv, 0, 0, plans[0][0][1][1])
                    nc.sync.dma_start(
                        out=w12_g0[0][1][:, :, :g0len * 128],
                        in_=w1t[0, h0:, :, :g0len * 128].rearrange("a p f -> p a f"),
                    )
                else:
                    for half in range(2):
                        src = wt_[0, half * h0:(half + 1) * h0, :, :g0len * 128]
                        nc.sync.dma_start(
                            out=halves[half][:, :, :g0len * 128],
                            in_=src.rearrange("a p f -> p a f"),
                        )
            w3_next = dma_w3(0, *groups[0])
            if n_sl > 1:
                dma_xg(0, xg, 1, plans[0][1][1][0], plans[0][1][1][1])
                dma_xv(0, xv, 1, plans[0][1][1][0], plans[0][1][1][1])
            for si, (s0, w) in plans[0][2:]:
                dma_xg(0, xg, si, s0, w)
                dma_xv(0, xv, si, s0, w)
            w12_next = None  # group0/slot0 uses the split tiles above

            for e in range(E_LOCAL):
                cap_e = caps[e]
                slices = plans[e]
                # per-slot y accumulator: slot0's writeout DMA can then drain
                # lazily under all of slot1's compute (no WAR deadline).
                y_acc = p_y.tile(
                    [128, DK, caps[e]], f32, tag=f"yacc{e}", name=f"yacc{e}"
                )

                for gi, (f0, glen) in enumerate(groups):
                    if e == 0 and gi == 0:
                        def w1sel(dk, fsl):
                            return w12_g0[0][dk // h0][:, dk % h0, fsl]

                        def w2sel(dk, fsl):
                            return w12_g0[1][dk // h0][:, dk % h0, fsl]
                    else:
                        w1r, w2r = w12_next

                        def w1sel(dk, fsl, _t=w1r):
                            return _t[:, dk, fsl]

                        def w2sel(dk, fsl, _t=w2r):
                            return _t[:, dk, fsl]
                    w3r = w3_next
                    # prefetch next group's weights (and slot1 preamble at the
                    # end of slot0's stream)
                    if gi + 1 < NG:
                        w12_next = dma_w12(e, *groups[gi + 1])
                        w3_next = dma_w3(e, *groups[gi + 1])
                        if e == 0 and gi + 2 == NG and big_early:
                            # slot1's big x slices: fresh buffers, no WAR
                            # wait, transfer while slot0 still has 2 groups
                            # to go.
                            s1, w1_ = plans[1][big_si][1]
                            xg1[big_si] = alloc_x("xg", big_si, plan0[big_si])
                            xv1[big_si] = alloc_x("xv", big_si, plan0[big_si])
                            dma_xg(1, xg1, big_si, s1, w1_)
                            dma_xv(1, xv1, big_si, s1, w1_)
                    elif e == 0:
                        w12_next = dma_w12(1, *groups[0])
                        w3_next = dma_w3(1, *groups[0])

                    # last group: biggest slice first, so its y writeout
                    # flushes under the remaining slices' compute and the
                    # kernel tail only carries the smallest slice.
                    g_slices = (
                        sorted(slices, key=lambda t: -t[1][1])
                        if gi == NG - 1 else slices
                    )
                    for si, (s0, w) in g_slices:
                        xgs = xg[si] if e == 0 else xg1[si]
                        xvs = xv[si] if e == 0 else xv1[si]
                        hs_tiles = []
                        for fl in range(glen):
                            pg = p_gv.tile([128, 512], f32, tag="gv")
                            for dk in range(DK):
                                nc.tensor.matmul(
                                    pg[:, :w],
                                    w1sel(dk, slice(fl * 128, (fl + 1) * 128)),
                                    xgs[:, dk, :w],
                                    start=(dk == 0),
                                    stop=(dk == DK - 1),
                                )
                            pv = p_gv.tile([128, 512], f32, tag="gv")
                            for dk in range(DK):
                                nc.tensor.matmul(
                                    pv[:, :w],
                                    w2sel(dk, slice(fl * 128, (fl + 1) * 128)),
                                    xvs[:, dk, :w],
                                    start=(dk == 0),
                                    stop=(dk == DK - 1),
                                )
                            sil = p_sil.tile([128, 512], f32, tag="sil")
                            nc.scalar.activation(
                                sil[:, :w], pg[:, :w], mybir.ActivationFunctionType.Silu
                            )
                            hst = p_hs.tile([128, 512], f32r, tag="hs")
                            nc.vector.tensor_mul(hst[:, :w], sil[:, :w], pv[:, :w])
                            hs_tiles.append(hst)

                        if (
                            e == 0 and gi == NG - 1 and si < len(plans[1])
                            and not (big_early and si == big_si)
                        ):
                            # slot0's g/v matmuls above were the last readers
                            # of these x slice tiles; refill with slot1's
                            # tokens now so the transfer hides under the rest
                            # of slot0's compute.
                            s1, w1_ = plans[1][si][1]
                            dma_xg(1, xg1, si, s1, w1_)
                            dma_xv(1, xv1, si, s1, w1_)

                        for di in range(DK):
                            py = p_py.tile([128, 512], f32, tag="py")
                            for fl in range(glen):
                                nc.tensor.matmul(
                                    py[:, :w],
                                    w3r[:, fl, di, :],
                                    hs_tiles[fl][:, :w],
                                    start=(fl == 0),
                                    stop=(fl == glen - 1),
                                )
                            if gi == 0:
                                nc.vector.tensor_copy(
                                    y_acc[:, di, s0:s0 + w], py[:, :w]
                                )
                            else:
                                nc.vector.tensor_add(
                                    y_acc[:, di, s0:s0 + w],
                                    y_acc[:, di, s0:s0 + w],
                                    py[:, :w],
                                )
                        if gi == NG - 1 and e == 1:
                            # slot1: per-slice writeout -- spread across the
                            # last group so the tail only flushes the smallest
                            # slice; that final slice goes out in two di-halves
                            # so its first half drains under the second half's
                            # adds.
                            if (si, (s0, w)) == g_slices[-1]:
                                for dlo, dhi in ((0, DK // 2), (DK // 2, DK)):
                                    nc.scalar.dma_start(
                                        out=yt[e][dlo:dhi, :, s0:s0 + w].rearrange(
                                            "a p f -> p a f"
                                        ),
                                        in_=y_acc[:, dlo:dhi, s0:s0 + w],
                                    )
                            else:
                                nc.scalar.dma_start(
                                    out=yt[e][:, :, s0:s0 + w].rearrange(
                                        "a p f -> p a f"
                                    ),
                                    in_=y_acc[:, :, s0:s0 + w],
                                )

                    if gi == NG - 1 and e == 0:
                        # slot0: one whole-tensor writeout; it drains lazily
                        # under slot1's compute (y_acc is per-slot, so no WAR
                        # deadline) and costs a single issue slot.
                        nc.scalar.dma_start(
                            out=yt[e][:, :, :].rearrange("a p f -> p a f"),
                            in_=y_acc[:, :, :cap_e],
                        )

    _split_excess_waits(nc)
    return nc


def _starts(plan: list[int]) -> list[tuple[int, int]]:
    out, s0 = [], 0
    for w in plan:
        out.append((s0, w))
        s0 += w
    return out


_BUILD_CACHE: dict[tuple, object] = {}


def _get_nc(cap0: int, cap1: int):
    key = (cap0, cap1)
    if key not in _BUILD_CACHE:
        _BUILD_CACHE[key] = _build(cap0, cap1)
    return _BUILD_CACHE[key]


# -------------------------------------------------------------------- kernel


def _fragment(counts, L):
    """Chop each expert's token run into fragments of <= L tokens.

    Returns a list of (expert, offset, length), or None if more than
    N_EXPERTS fragments would be needed (one device slot each)."""
    frags = []
    for g in range(N_EXPERTS):
        c, off = int(counts[g]), 0
        while c > 0:
            ln = min(L, c)
            frags.append((g, off, ln))
            off += ln
            c -= ln
            if len(frags) > N_EXPERTS:
                return None
    return frags


def prepare(x, expert_indices, expert_weights, w1, w2, w3):
    """Host routing + sharding. Returns (nc, in_maps, meta)."""
    x = np.asarray(x)
    ei = np.asarray(expert_indices).reshape(-1)
    ew = np.asarray(expert_weights).reshape(-1).astype(np.float32)
    w1 = np.asarray(w1)
    w2 = np.asarray(w2)
    w3 = np.asarray(w3)

    # ---- integer routing on host (replicated bookkeeping)
    order = np.argsort(ei, kind="stable")
    tok_sorted = (np.repeat(np.arange(N_TOKENS, dtype=np.int64), TOP_K))[order]
    w_sorted = ew[order]
    counts = np.bincount(ei, minlength=N_EXPERTS)
    seg = np.concatenate(([0], np.cumsum(counts)))

    # ---- choose slot fragments: one fragment per expert when balanced;
    # under heavy skew, split oversized experts across slots (weights are
    # duplicated per slot) so the static caps fit SBUF. Largest-with-smallest
    # pairing keeps cap0/cap1 minimal for the SPMD one-program constraint.
    total = int(counts.sum())
    L_floor = max(1, -(-total // N_EXPERTS))
    L = max(1, int(counts.max()))
    nc = last_err = None
    while True:
        frags = _fragment(counts, L)
        if frags is None:
            if last_err is not None:
                raise last_err
            raise RuntimeError("cannot fit expert fragments into slots")
        frags = sorted(frags, key=lambda f: -f[2])
        frags += [(0, 0, 0)] * (N_EXPERTS - len(frags))
        cap0 = max(256, -(-frags[0][2] // 8) * 8)
        cap1 = max(256, -(-frags[N_CORES][2] // 8) * 8)
        try:
            nc = _get_nc(cap0, cap1)
            break
        except ValueError as err:  # SBUF overflow: shrink fragments
            last_err = err
        if L <= L_floor:
            raise last_err
        L = max(L_floor, (L * 3) // 4)
    pairs = [(frags[c], frags[N_EXPERTS - 1 - c]) for c in range(N_CORES)]

    # ---- round matmul operands to the device matmul dtype on host
    if _MM_DT == "bfloat16":
        import ml_dtypes

        mm_np = ml_dtypes.bfloat16
        xr = x.astype(mm_np)
        w1r = w1.astype(mm_np)
        w2r = w2.astype(mm_np)
        w3r = w3.astype(mm_np)
    else:
        mm_np = np.float32
        xr = _to_f32r(x)
        w1r = _to_f32r(w1)
        w2r = _to_f32r(w2)
        w3r = _to_f32r(w3)

    caps = [cap0, cap1]
    in_maps = []
    for c in range(N_CORES):
        m = {}
        w1c = np.empty((E_LOCAL, DK, 128, D_FF), dtype=mm_np)
        w2c = np.empty((E_LOCAL, DK, 128, D_FF), dtype=mm_np)
        w3c = np.empty((E_LOCAL, FK, 128, D_MODEL), dtype=mm_np)
        for e in range(E_LOCAL):
            g, off, ln = pairs[c][e]
            lo = seg[g] + off
            toks = tok_sorted[lo:lo + ln]
            xt_c = np.zeros((D_MODEL, caps[e]), dtype=mm_np)
            xt_c[:, :ln] = xr[toks].T
            # value-path tokens pre-scaled by the routing weight: the w3
            # contraction then emits route-weighted y directly.
            xv_c = np.zeros((D_MODEL, caps[e]), dtype=mm_np)
            xv_c[:, :ln] = (
                x[toks].astype(np.float32) * w_sorted[lo:lo + ln, None]
            ).T.astype(mm_np)
            m[f"xt{e}"] = xt_c.reshape(DK, 128, caps[e])
            m[f"xv{e}"] = xv_c.reshape(DK, 128, caps[e])
            w1c[e] = w1r[g].reshape(DK, 128, D_FF)
            w2c[e] = w2r[g].reshape(DK, 128, D_FF)
            w3c[e] = w3r[g].reshape(FK, 128, D_MODEL)
        m["w1t"] = w1c
        m["w2t"] = w2c
        m["w3t"] = w3c
        in_maps.append(m)

    meta = {"seg": seg, "tok_sorted": tok_sorted, "pairs": pairs, "caps": caps}
    return nc, in_maps, meta


def combine(results, meta):
    """Unshard per-core fragment outputs and sum the top-2 contributions."""
    seg, tok_sorted = meta["seg"], meta["tok_sorted"]
    pairs, caps = meta["pairs"], meta["caps"]
    assign_rows = np.empty((N_TOKENS * TOP_K, D_MODEL), dtype=np.float32)
    for c in range(N_CORES):
        for e in range(E_LOCAL):
            g, off, ln = pairs[c][e]
            lo = seg[g] + off
            ytc = results[c][f"yt{e}"].reshape(D_MODEL, caps[e])
            assign_rows[lo:lo + ln] = ytc[:, :ln].T

    by_token = np.argsort(tok_sorted, kind="stable")
    out = assign_rows[by_token].reshape(N_TOKENS, TOP_K, D_MODEL).sum(axis=1)
    return out.astype(np.float32)


def kernel(x, expert_indices, expert_weights, w1, w2, w3, _run_opts=None):
    from concourse.bass_utils import run_bass_kernel_spmd

    nc, in_maps, meta = prepare(x, expert_indices, expert_weights, w1, w2, w3)
    opts = dict(_run_opts or {})
    res = run_bass_kernel_spmd(nc, in_maps, list(range(N_CORES)), **opts)
    if _run_opts is not None:
        _run_opts["result"] = res
    return combine(res.results, meta)

